# revision 1
# baseline (speedup 1.0000x reference)
"""Trainium2 Bass kernel v2 for GQA attention (nn_Attention_50053548868012).

Deltas vs v1 (kernel.py):
 - all matmul operands bf16 (same PE rate as f32r on this rig, but halves
   every DMA and SBUF footprint, and enables FWL weight loads)
 - phase 1: MB=16 two-block structure, full 16-deep PSUM accumulation per
   block (one SBUF add instead of three), per-chunk RoPE emitted inline so
   DVE/DMA rope work overlaps the next chunk's matmuls (kills the ~45us
   PE stall between phases 1 and 2)
 - causal mask generated on device (gpsimd affine_select), no mask input
 - output written bf16 (host upcasts and sums the 8 partials)
"""

import numpy as np
import ml_dtypes

import concourse.bass as bass
import concourse.tile as tile
from concourse import bacc, mybir
from concourse.bass_utils import run_bass_kernel_spmd
from concourse.masks import make_identity

NCORES = 8
S = 2048
MD = 4096
H = 128
R = 4
KV = 8
PT = 128          # partition tile
SC = 512          # free-dim chunk
RH = R * H        # 512
MB = 16           # m-tiles per phase-1 block
SCALE = float(H) ** -0.5
NEG = -30000.0

f32 = mybir.dt.float32
f32r = mybir.dt.float32r
bf16 = mybir.dt.bfloat16
BFNP = ml_dtypes.bfloat16


def build_bass(s=S, collective=True, phases=3, reps=1):
    nc = _emit(s, collective, phases, reps)
    nc.compile()
    return nc


def _emit(s, collective, phases, reps=1):
    assert s % SC == 0
    n_sc = s // SC          # seq chunks (4)
    n_mt = MD // PT         # model-dim tiles (32)
    n_tt = s // PT          # seq tiles of 128 (16)
    tpc = SC // PT          # 128-tiles per chunk (4)
    n_blk = n_mt // MB      # phase-1 m-blocks (2)
    hh = H // 2

    nc = bacc.Bacc("TRN2", target_bir_lowering=False, debug=False,
                   num_devices=NCORES)

    xT = nc.dram_tensor("xT", [PT, n_mt, s], bf16, kind="ExternalInput").ap()
    wq = nc.dram_tensor("wq", [PT, n_mt, RH], bf16, kind="ExternalInput").ap()
    wk = nc.dram_tensor("wk", [PT, n_mt, H], bf16, kind="ExternalInput").ap()
    wv = nc.dram_tensor("wv", [PT, n_mt, H], bf16, kind="ExternalInput").ap()
    wo = nc.dram_tensor("wo", [PT, R, MD], bf16, kind="ExternalInput").ap()
    cosT = nc.dram_tensor("cosT", [H, s], bf16, kind="ExternalInput").ap()
    sinT = nc.dram_tensor("sinT", [H, s], bf16, kind="ExternalInput").ap()
    outp = nc.dram_tensor("outp", [PT, n_tt, MD], bf16,
                          kind="ExternalOutput").ap()

    with tile.TileContext(nc) as tc:
      for _rep in range(reps):
        with tc.tile_pool(name="const", bufs=1) as const_pool, \
             tc.tile_pool(name="dram", bufs=1, space="DRAM") as dram_pool:
            ones_f = const_pool.tile([PT, PT], f32)
            nc.gpsimd.memset(ones_f[:], 1.0)
            ones_b = const_pool.tile([PT, PT], bf16)
            nc.scalar.copy(ones_b[:], ones_f[:])
            ones_r = const_pool.tile([PT, PT], f32r)
            nc.scalar.copy(ones_r[:], ones_f[:])
            ident = const_pool.tile([PT, PT], bf16)
            make_identity(nc, ident[:])
            # causal mask for the diagonal chunk: mask[p, j, q] = 0 where
            # 128*j + p <= q else NEG  (j = k-tile within chunk, q in chunk)
            mask_sb = const_pool.tile([PT, tpc, SC], bf16)
            nc.gpsimd.memset(mask_sb[:], 0.0)
            nc.gpsimd.affine_select(
                out=mask_sb[:], in_=mask_sb[:],
                compare_op=mybir.AluOpType.is_ge, fill=NEG, base=0,
                pattern=[[-PT, tpc], [1, SC]], channel_multiplier=-1)
            cos_sb = const_pool.tile([H, s], bf16)
            nc.gpsimd.dma_start(cos_sb[:], cosT)
            sin_sb = const_pool.tile([H, s], bf16)
            nc.gpsimd.dma_start(sin_sb[:], sinT)

            persist_ctx = tc.tile_pool(name="persist", bufs=1)
            persist = persist_ctx.__enter__()
            qT_sb = persist.tile([H, R, s], bf16)
            kT_sb = persist.tile([H, s], bf16)
            v_sb = persist.tile([PT, n_tt, H], bf16)
            yT_sb = persist.tile([H, R, s], bf16)

            # ---------- Phase 1: projections (2 m-blocks) + inline RoPE ----
            with tc.tile_pool(name="ph1", bufs=1) as ph1_pool, \
                 tc.tile_pool(name="w1", bufs=2) as w_pool, \
                 tc.tile_pool(name="xslab", bufs=1) as x_pool, \
                 tc.tile_pool(name="rope", bufs=2) as rope_pool, \
                 tc.tile_pool(name="p1ps", bufs=1, space="PSUM") as p1_psum, \
                 tc.tile_pool(name="tpps", bufs=2, space="PSUM") as tp_psum:
                # per-chunk pre-rope accumulators: separate tiles so chunk
                # c+1's spills don't serialize behind chunk c's rope
                # (tile-granular dependency tracking)
                qacc = [ph1_pool.tile([H, R, SC], bf16, tag=f"qa{c}",
                                      name=f"qa{c}") for c in range(n_sc)]
                kacc = [ph1_pool.tile([H, SC], bf16, tag=f"ka{c}",
                                      name=f"ka{c}") for c in range(n_sc)]
                vacc = [ph1_pool.tile([H, SC], bf16, tag=f"va{c}",
                                      name=f"va{c}") for c in range(n_sc)]
                pending_rope = []

                for blk in range(n_blk):
                    # k/v weights + first x slab first so the PE can start
                    # ~2us in; wq (2MB) streams under the first k/v matmuls
                    xs = []
                    x0 = x_pool.tile([PT, s], bf16, tag="x0", name="x0")
                    nc.sync.dma_start(x0[:], xT[:, blk * MB, :])
                    xs.append(x0)
                    wk_b = w_pool.tile([PT, MB, H], bf16, tag="wkb",
                                       name="wk_b")
                    nc.sync.dma_start(
                        wk_b[:], wk[:, blk * MB:(blk + 1) * MB, :])
                    wv_b = w_pool.tile([PT, MB, H], bf16, tag="wvb",
                                       name="wv_b")
                    nc.sync.dma_start(
                        wv_b[:], wv[:, blk * MB:(blk + 1) * MB, :])
                    wq_b = w_pool.tile([PT, MB, RH], bf16, tag="wqb",
                                       name="wq_b")
                    nc.sync.dma_start(
                        wq_b[:], wq[:, blk * MB:(blk + 1) * MB, :])
                    for ml in range(1, MB):
                        xsl = x_pool.tile([PT, s], bf16, tag=f"x{ml}",
                                          name="xsl")
                        nc.sync.dma_start(xsl[:], xT[:, blk * MB + ml, :])
                        xs.append(xsl)
                    for sc_i in range(n_sc):
                        ssl = slice(sc_i * SC, (sc_i + 1) * SC)
                        ps6 = [p1_psum.tile([PT, SC], f32, tag=f"pa{u}",
                                            name=f"ps6_{u}")
                               for u in range(R + 2)]
                        for ml in range(MB):
                            rx = xs[ml][:, ssl]
                            st = ml == 0
                            sp = ml == MB - 1
                            nc.tensor.matmul(
                                ps6[R][:], wk_b[:, ml, :], rx,
                                start=st, stop=sp)
                            nc.tensor.matmul(
                                ps6[R + 1][:], wv_b[:, ml, :], rx,
                                start=st, stop=sp)
                            for j in range(R):
                                nc.tensor.matmul(
                                    ps6[j][:],
                                    wq_b[:, ml, j * H:(j + 1) * H],
                                    rx, start=st, stop=sp)
                        accs = ([qacc[sc_i][:, j, :] for j in range(R)]
                                + [kacc[sc_i][:], vacc[sc_i][:]])
                        for u in range(R + 2):
                            if blk == 0:
                                nc.scalar.copy(accs[u], ps6[u][:])
                            else:
                                nc.vector.tensor_add(
                                    accs[u], ps6[u][:], accs[u])
                        if blk == n_blk - 1:
                            # swap-DMAs issue now (Pool/SWDGE, overlapped);
                            # the DVE rope math is DEFERRED one chunk so it
                            # queues BEHIND the next chunk's spill-adds in
                            # the DVE FIFO (bank release stays prompt)
                            qsw = rope_pool.tile([H, R, SC], bf16, tag="qsw",
                                                 name="qsw")
                            nc.gpsimd.dma_start(
                                qsw[0:hh, :, :], qacc[sc_i][hh:H, :, :])
                            nc.gpsimd.dma_start(
                                qsw[hh:H, :, :], qacc[sc_i][0:hh, :, :])
                            ksw = rope_pool.tile([H, SC], bf16, tag="ksw",
                                                 name="ksw")
                            nc.gpsimd.dma_start(
                                ksw[0:hh, :], kacc[sc_i][hh:H, :])
                            nc.gpsimd.dma_start(
                                ksw[hh:H, :], kacc[sc_i][0:hh, :])

                            def rope_math(sc_i=sc_i, ssl=ssl, qsw=qsw,
                                          ksw=ksw):
                                # per-head ops: no broadcast APs (keeps DVE
                                # fast mode), finer FIFO interleave
                                for j in range(R):
                                    nc.vector.tensor_mul(
                                        qsw[:, j, :], qsw[:, j, :],
                                        sin_sb[:, ssl])
                                    nc.vector.tensor_mul(
                                        qT_sb[:, j, ssl],
                                        qacc[sc_i][:, j, :],
                                        cos_sb[:, ssl])
                                    nc.vector.tensor_add(
                                        qT_sb[:, j, ssl], qT_sb[:, j, ssl],
                                        qsw[:, j, :])
                                nc.vector.tensor_mul(
                                    ksw[:], ksw[:], sin_sb[:, ssl])
                                nc.vector.tensor_mul(
                                    kT_sb[:, ssl], kacc[sc_i][:],
                                    cos_sb[:, ssl])
                                nc.vector.tensor_add(
                                    kT_sb[:, ssl], kT_sb[:, ssl], ksw[:])
                                for tl in range(tpc):
                                    tt = sc_i * tpc + tl
                                    ps_t = tp_psum.tile([PT, PT], bf16,
                                                        tag="tp",
                                                        name="ps_t")
                                    nc.tensor.transpose(
                                        ps_t[:],
                                        vacc[sc_i][:, tl * PT:(tl + 1) * PT],
                                        ident[:])
                                    nc.scalar.copy(v_sb[:, tt, :], ps_t[:])

                            pending_rope.append(rope_math)
                            if len(pending_rope) > 1:
                                pending_rope.pop(0)()
                while pending_rope:
                    pending_rope.pop(0)()

            if phases >= 2:
              # wo pool spans phases 2+3 (prefetch overlaps attention)
              with tc.tile_pool(name="w3", bufs=1) as w3_pool:
                wo_sb = w3_pool.tile([PT, R, MD], bf16)
                for rl in range(R):
                    nc.sync.dma_start(wo_sb[:, rl, :], wo[:, rl, :])

                # ------- Phase 2: attention (+ interleaved phase 3) -------
                n_mc = MD // RH
                with tc.tile_pool(name="epool", bufs=6) as e_pool, \
                     tc.tile_pool(name="zpool", bufs=2) as z_pool, \
                     tc.tile_pool(name="osb", bufs=3) as o_pool, \
                     tc.tile_pool(name="p2ps", bufs=2, space="PSUM") as p2_psum:
                    pending_fin = []
                    pending_ph3 = []

                    def emit_ph3(c):
                        # output projection for the 4 seq-tiles of chunk c
                        # (dense, always-ready PE filler between chunks)
                        for tl in range(tpc):
                            st = c * tpc + tl
                            o_acc = o_pool.tile([PT, MD], bf16, tag="oacc",
                                                name="o_acc")
                            for mc in range(n_mc):
                                ps_o = p2_psum.tile(
                                    [PT, RH], f32, tag="po", bufs=2,
                                    name="ps_o")
                                for rl in range(R):
                                    nc.tensor.matmul(
                                        ps_o[:],
                                        yT_sb[:, rl,
                                              st * PT:(st + 1) * PT],
                                        wo_sb[:, rl,
                                              mc * RH:(mc + 1) * RH],
                                        start=(rl == 0),
                                        stop=(rl == R - 1))
                                if mc % 2 == 0:
                                    nc.scalar.copy(
                                        o_acc[:, mc * RH:(mc + 1) * RH],
                                        ps_o[:])
                                else:
                                    nc.vector.tensor_copy(
                                        o_acc[:, mc * RH:(mc + 1) * RH],
                                        ps_o[:])
                            nc.sync.dma_start(outp[:, st, :], o_acc[:])
                    for c in range(n_sc):
                        T = (c + 1) * tpc
                        csl = slice(c * SC, (c + 1) * SC)
                        for j in range(R):
                            ps_y = p2_psum.tile([H, SC], f32, tag="py",
                                                bufs=1, name="ps_y")
                            ps_z = p2_psum.tile([1, SC], f32, tag="pz",
                                                bufs=1, name="ps_z")
                            rq = qT_sb[:, j, csl]
                            es = {}

                            def qk_exp(p, rq=rq, T=T, es=es):
                                t0 = 2 * p
                                ps_s = p2_psum.tile([PT, 2 * SC], f32,
                                                    tag="ps", bufs=2,
                                                    name="ps_s")
                                nc.tensor.matmul(
                                    ps_s[:, 0:SC],
                                    kT_sb[:, t0 * PT:(t0 + 1) * PT],
                                    rq, start=True, stop=True)
                                nc.tensor.matmul(
                                    ps_s[:, SC:2 * SC],
                                    kT_sb[:, (t0 + 1) * PT:(t0 + 2) * PT],
                                    rq, start=True, stop=True)
                                e_t = e_pool.tile([PT, 2 * SC], bf16,
                                                  tag="e", name="e_t")
                                nc.scalar.activation(
                                    e_t[:], ps_s[:],
                                    mybir.ActivationFunctionType.Exp,
                                    scale=SCALE)
                                dt0 = t0 - (T - tpc)
                                if dt0 >= 0:
                                    # zero the causally-masked region on the
                                    # idle Pool engine: keep where
                                    # q - p - 128*(dt0 + a) >= 0
                                    ev = e_t[:].rearrange(
                                        "k (a b) -> k a b", a=2)
                                    nc.gpsimd.affine_select(
                                        out=ev, in_=ev,
                                        compare_op=mybir.AluOpType.is_ge,
                                        fill=0.0, base=-PT * dt0,
                                        pattern=[[-PT, 2], [1, SC]],
                                        channel_multiplier=-1)
                                es[p] = e_t

                            P2 = T // 2
                            # diagonal pair first: its DVE mask-add + exp
                            # latency hides under the previous head's AV
                            # tail instead of stalling this head's tail
                            order = [P2 - 1, P2 - 2] + list(range(P2 - 2))
                            qk_exp(order[0])
                            while pending_fin:
                                pending_fin.pop(0)()
                            # z matmuls deferred one pair behind the AV
                            # matmuls: consecutive `ones` stationary loads
                            # (no weight-buffer thrash) and always-ready
                            # PE filler during exp waits
                            ny = 0
                            nz = 0
                            zq = []
                            for oi, p in enumerate(order):
                                if oi + 1 < P2:
                                    qk_exp(order[oi + 1])
                                e_t = es.pop(p)
                                for half in range(2):
                                    t = 2 * p + half
                                    esl = slice(half * SC, (half + 1) * SC)
                                    nc.tensor.matmul(
                                        ps_y[:], v_sb[:, t, :],
                                        e_t[:, esl],
                                        start=(ny == 0),
                                        stop=(ny == T - 1))
                                    ny += 1
                                    zq.append(e_t[:, esl])
                                if oi > 0:
                                    while len(zq) > 2:
                                        ez = zq.pop(0)
                                        nc.tensor.matmul(
                                            ps_z[:], ones_b[:, 0:1], ez,
                                            start=(nz == 0),
                                            stop=(nz == T - 1))
                                        nz += 1
                            while zq:
                                ez = zq.pop(0)
                                nc.tensor.matmul(
                                    ps_z[:], ones_b[:, 0:1], ez,
                                    start=(nz == 0), stop=(nz == T - 1))
                                nz += 1

                            def finalize(c=c, j=j, ps_y=ps_y, ps_z=ps_z,
                                         csl=csl):
                                rz = z_pool.tile([1, SC], f32, tag="rz",
                                                 name="rz")
                                with nc.allow_low_precision(
                                        reason="full-width recip"):
                                    nc.vector.reciprocal(rz[:], ps_z[:])
                                # broadcast 1/z across partitions on the
                                # (idle) Pool engine instead of PE+ACT
                                b_sb = z_pool.tile([PT, SC], f32,
                                                   tag="bsb", name="b_sb")
                                nc.gpsimd.partition_broadcast(
                                    b_sb[:], rz[:])
                                nc.vector.tensor_mul(
                                    yT_sb[:, j, csl], ps_y[:], b_sb[:])

                            pending_fin.append(finalize)
                        if phases >= 3:
                            pending_ph3.append(c)
                            if len(pending_ph3) > 1:
                                emit_ph3(pending_ph3.pop(0))
                    while pending_fin:
                        pending_fin.pop(0)()
                    while pending_ph3:
                        emit_ph3(pending_ph3.pop(0))

            persist_ctx.__exit__(None, None, None)
    return nc


def _pack_pm(a):
    """[n_mt*128, C] -> [128, n_mt, C] partition-major, bf16."""
    n_mt = a.shape[0] // PT
    return np.ascontiguousarray(
        a.reshape(n_mt, PT, a.shape[1]).transpose(1, 0, 2)).astype(BFNP)


def shard_inputs(x, wq, wk, wv, wo, mask, sin, cos, s=S):
    del mask  # causality generated on device
    xTp = _pack_pm(np.ascontiguousarray(
        np.asarray(x, dtype=np.float32).reshape(s, MD).T))
    cosT = np.ascontiguousarray(
        np.asarray(cos, dtype=np.float32).T).astype(BFNP)
    sign = np.concatenate(
        [-np.ones((H // 2, 1)), np.ones((H // 2, 1))]).astype(np.float32)
    sinTs = np.ascontiguousarray(
        np.asarray(sin, dtype=np.float32).T * sign).astype(BFNP)
    wo = np.asarray(wo, dtype=np.float32)
    wq = np.asarray(wq, dtype=np.float32)
    wk = np.asarray(wk, dtype=np.float32)
    wv = np.asarray(wv, dtype=np.float32)
    in_maps = []
    for c in range(NCORES):
        in_maps.append({
            "xT": xTp,
            "wq": _pack_pm(np.ascontiguousarray(
                wq[:, :, c, :].reshape(MD, RH))),
            "wk": _pack_pm(np.ascontiguousarray(wk[:, c, :])),
            "wv": _pack_pm(np.ascontiguousarray(wv[:, c, :])),
            "wo": _pack_pm(np.ascontiguousarray(
                wo[:, c, :, :].reshape(RH, MD))),
            "cosT": cosT,
            "sinT": sinTs,
        })
    return in_maps


def unpack_out(outp_arr, s=S):
    """[128, s/128, MD] bf16 -> [s, MD] f32."""
    return np.ascontiguousarray(
        np.asarray(outp_arr).astype(np.float32).reshape(
            PT, s // PT, MD).transpose(1, 0, 2).reshape(s, MD))


_NC_CACHE = {}


def kernel(x, wq, wk, wv, wo, mask, sin, cos):
    s = x.shape[1]
    if s not in _NC_CACHE:
        _NC_CACHE[s] = build_bass(s)
    nc = _NC_CACHE[s]
    in_maps = shard_inputs(x, wq, wk, wv, wo, mask, sin, cos, s=s)
    res = run_bass_kernel_spmd(nc, in_maps, list(range(NCORES)))
    out = unpack_out(res.results[0]["outp"], s)
    for c in range(1, NCORES):
        out = out + unpack_out(res.results[c]["outp"], s)
    return out.reshape(1, s, MD).astype(np.float32)



# revision 3
# speedup vs baseline: 1.0689x; 1.0689x over previous
"""Trainium2 Bass kernel v4 for GQA attention (nn_Attention_50053548868012).

Deltas vs v3 (driven by phase ablation: p1=235us, p2=+185us, p3=+135us,
sum == total, i.e. the between-chunk ph3 interleave filled nothing
because the PE queue is strict FIFO):
 - phase 3 matmul groups are injected at TILE granularity inside the
   attention loops, so the PE always has ready work queued between a
   QK matmul and the AV matmul that waits on its exp.
 - QK/exp/AV restructured from pair (two-bank [128,1024]) to single-tile
   [128,512] granularity: shorter exp latency quantum (720ns vs 1147ns)
   in the QK->exp->AV chain, and ps_s fits 3 single banks (freeing one
   bank so ps_z gets its own and ph3's ps_o keeps 2).
 - phase-1 DMA ordering: first x sub-slab issued before the wq halves
   and x goes on the scalar HWDGE ring (weights on sync ring) so the
   first matmul group isn't queued behind 6MB of weights.
 - phase-1 spills split ACT/DVE to halve the chunk-boundary PSUM-bank
   serialization.
"""

import numpy as np
import ml_dtypes

import concourse.bass as bass
import concourse.tile as tile
from concourse import bacc, mybir
from concourse.bass_utils import run_bass_kernel_spmd
from concourse.masks import make_identity

NCORES = 8
S = 2048
MD = 4096
H = 128
R = 4
KV = 8
PT = 128          # partition tile
SC = 512          # free-dim chunk
RH = R * H        # 512
SCALE = float(H) ** -0.5
NEG = -30000.0

f32 = mybir.dt.float32
bf16 = mybir.dt.bfloat16
BFNP = ml_dtypes.bfloat16


def build_bass(s=S, collective=True, phases=3, reps=1):
    nc = _emit(s, collective, phases, reps)
    nc.compile()
    return nc


def _emit(s, collective, phases, reps=1):
    assert s % SC == 0
    n_sc = s // SC          # seq chunks (4)
    n_mt = MD // PT         # model-dim tiles (32)
    n_tt = s // PT          # seq tiles of 128 (16)
    tpc = SC // PT          # 128-tiles per chunk (4)
    hh = H // 2

    nc = bacc.Bacc("TRN2", target_bir_lowering=False, debug=False,
                   num_devices=NCORES)

    # x chunk-major: [128, chunk, m_tile * 512] so one chunk slab is a
    # single contiguous DMA
    xc = nc.dram_tensor("xc", [PT, n_sc, n_mt * SC], bf16,
                        kind="ExternalInput").ap()
    wq = nc.dram_tensor("wq", [PT, n_mt, RH], bf16, kind="ExternalInput").ap()
    wk = nc.dram_tensor("wk", [PT, n_mt, H], bf16, kind="ExternalInput").ap()
    wv = nc.dram_tensor("wv", [PT, n_mt, H], bf16, kind="ExternalInput").ap()
    wo = nc.dram_tensor("wo", [PT, R, MD], bf16, kind="ExternalInput").ap()
    cosT = nc.dram_tensor("cosT", [H, s], bf16, kind="ExternalInput").ap()
    sinT = nc.dram_tensor("sinT", [H, s], bf16, kind="ExternalInput").ap()
    outp = nc.dram_tensor("outp", [PT, n_tt, MD], bf16,
                          kind="ExternalOutput").ap()

    with tile.TileContext(nc) as tc:
      for _rep in range(reps):
        with tc.tile_pool(name="const", bufs=1) as const_pool:
            ones_f = const_pool.tile([PT, PT], f32)
            nc.gpsimd.memset(ones_f[:], 1.0)
            ones_b = const_pool.tile([PT, PT], bf16)
            nc.scalar.copy(ones_b[:], ones_f[:])
            ident = const_pool.tile([PT, PT], bf16)
            make_identity(nc, ident[:])
            cos_sb = const_pool.tile([H, s], bf16)
            nc.gpsimd.dma_start(cos_sb[:], cosT)
            sin_sb = const_pool.tile([H, s], bf16)
            nc.gpsimd.dma_start(sin_sb[:], sinT)
            # 0/1 causal masks for the 4 diagonal-block offsets:
            # mask4[p, dt, q] = 1 where q >= p + 128*dt else 0
            mask4 = const_pool.tile([PT, tpc, SC], bf16)
            nc.gpsimd.memset(mask4[:], 1.0)
            nc.gpsimd.affine_select(
                out=mask4[:], in_=mask4[:],
                compare_op=mybir.AluOpType.is_ge, fill=0.0, base=0,
                pattern=[[-PT, tpc], [1, SC]], channel_multiplier=-1)

            persist_ctx = tc.tile_pool(name="persist", bufs=1)
            persist = persist_ctx.__enter__()
            qT_sb = persist.tile([H, R, s], bf16)
            kT_sb = persist.tile([H, s], bf16)
            v_sb = persist.tile([PT, n_tt, H], bf16)
            yT_sb = persist.tile([H, R, s], bf16)

            # ---------- Phase 1: projections (chunk-outer) + inline RoPE --
            with tc.tile_pool(name="w1", bufs=1) as w_pool, \
                 tc.tile_pool(name="xslab", bufs=2) as x_pool, \
                 tc.tile_pool(name="acc1", bufs=2) as acc_pool, \
                 tc.tile_pool(name="rope", bufs=2) as rope_pool, \
                 tc.tile_pool(name="p1ps", bufs=1, space="PSUM") as p1_psum, \
                 tc.tile_pool(name="tpps", bufs=2, space="PSUM") as tp_psum:
                # k/v weights first so the PE can start early; wq streams
                # under the first k/v matmuls.  x slabs go on the scalar
                # HWDGE ring so they aren't queued behind the weights.
                wk_b = w_pool.tile([PT, n_mt, H], bf16)
                nc.sync.dma_start(wk_b[:], wk)
                wv_b = w_pool.tile([PT, n_mt, H], bf16)
                nc.sync.dma_start(wv_b[:], wv)
                wq_b = w_pool.tile([PT, n_mt, RH], bf16)
                for q4 in range(4):
                    msl = slice(q4 * (n_mt // 4), (q4 + 1) * (n_mt // 4))
                    nc.sync.dma_start(wq_b[:, msl, :], wq[:, msl, :])
                pending_rope = []

                for sc_i in range(n_sc):
                    ssl = slice(sc_i * SC, (sc_i + 1) * SC)
                    xs = x_pool.tile([PT, n_mt, SC], bf16, tag="xs",
                                     name="xs")
                    # split the 4MB chunk load so matmuls start after 1MB
                    for q4 in range(4):
                        msl = slice(q4 * (n_mt // 4), (q4 + 1) * (n_mt // 4))
                        nc.sync.dma_start(xs[:, msl, :], xc[:, sc_i, :]
                                          .rearrange("p (m c) -> p m c",
                                                     m=n_mt)[:, msl, :])
                    ps6 = [p1_psum.tile([PT, SC], f32, tag=f"pa{u}",
                                        name=f"ps6_{u}")
                           for u in range(R + 2)]
                    for ml in range(n_mt):
                        rx = xs[:, ml, :]
                        st = ml == 0
                        sp = ml == n_mt - 1
                        nc.tensor.matmul(
                            ps6[R][:], wk_b[:, ml, :], rx, start=st, stop=sp)
                        nc.tensor.matmul(
                            ps6[R + 1][:], wv_b[:, ml, :], rx,
                            start=st, stop=sp)
                        for j in range(R):
                            nc.tensor.matmul(
                                ps6[j][:], wq_b[:, ml, j * H:(j + 1) * H],
                                rx, start=st, stop=sp)
                    # single spill PSUM->SBUF on ACT (idle in phase 1)
                    qacc = acc_pool.tile([H, R, SC], bf16, tag="qa",
                                         name="qacc")
                    kacc = acc_pool.tile([H, SC], bf16, tag="ka",
                                         name="kacc")
                    vacc = acc_pool.tile([H, SC], bf16, tag="va",
                                         name="vacc")
                    # split spills ACT/DVE so the 6 PSUM banks free in
                    # ~half the serial time at the chunk boundary
                    nc.scalar.copy(qacc[:, 0, :], ps6[0][:])
                    nc.vector.tensor_copy(qacc[:, 1, :], ps6[1][:])
                    nc.scalar.copy(qacc[:, 2, :], ps6[2][:])
                    nc.vector.tensor_copy(qacc[:, 3, :], ps6[3][:])
                    nc.scalar.copy(kacc[:], ps6[R][:])
                    nc.vector.tensor_copy(vacc[:], ps6[R + 1][:])

                    # rope swap-halves via Pool SWDGE (overlapped); the DVE
                    # rope math is DEFERRED one chunk so it queues behind
                    # the next chunk's matmul stream
                    qsw = rope_pool.tile([H, R, SC], bf16, tag="qsw",
                                         name="qsw")
                    nc.gpsimd.dma_start(qsw[0:hh, :, :], qacc[hh:H, :, :])
                    nc.gpsimd.dma_start(qsw[hh:H, :, :], qacc[0:hh, :, :])
                    ksw = rope_pool.tile([H, SC], bf16, tag="ksw",
                                         name="ksw")
                    nc.gpsimd.dma_start(ksw[0:hh, :], kacc[hh:H, :])
                    nc.gpsimd.dma_start(ksw[hh:H, :], kacc[0:hh, :])

                    def rope_math(sc_i=sc_i, ssl=ssl, qsw=qsw, ksw=ksw,
                                  qacc=qacc, kacc=kacc, vacc=vacc):
                        for j in range(R):
                            nc.vector.tensor_mul(
                                qsw[:, j, :], qsw[:, j, :], sin_sb[:, ssl])
                            nc.vector.tensor_mul(
                                qT_sb[:, j, ssl], qacc[:, j, :],
                                cos_sb[:, ssl])
                            nc.vector.tensor_add(
                                qT_sb[:, j, ssl], qT_sb[:, j, ssl],
                                qsw[:, j, :])
                        nc.vector.tensor_mul(ksw[:], ksw[:], sin_sb[:, ssl])
                        nc.vector.tensor_mul(
                            kT_sb[:, ssl], kacc[:], cos_sb[:, ssl])
                        nc.vector.tensor_add(
                            kT_sb[:, ssl], kT_sb[:, ssl], ksw[:])
                        for tl in range(tpc):
                            tt = sc_i * tpc + tl
                            ps_t = tp_psum.tile([PT, PT], bf16, tag="tp",
                                                name="ps_t")
                            nc.tensor.transpose(
                                ps_t[:], vacc[:, tl * PT:(tl + 1) * PT],
                                ident[:])
                            nc.scalar.copy(v_sb[:, tt, :], ps_t[:])

                    pending_rope.append(rope_math)
                    if len(pending_rope) > 1:
                        pending_rope.pop(0)()
                while pending_rope:
                    pending_rope.pop(0)()

            if phases >= 2:
              # wo pool spans phases 2+3 (prefetch overlaps attention)
              with tc.tile_pool(name="w3", bufs=1) as w3_pool:
                wo_sb = w3_pool.tile([PT, R, MD], bf16)
                for rl in range(R):
                    nc.sync.dma_start(wo_sb[:, rl, :], wo[:, rl, :])

                # ------- Phase 2: attention (+ tile-level phase 3) -------
                n_mc = MD // RH
                with tc.tile_pool(name="epool", bufs=8) as e_pool, \
                     tc.tile_pool(name="zpool", bufs=2) as z_pool, \
                     tc.tile_pool(name="osb", bufs=3) as o_pool, \
                     tc.tile_pool(name="p2ps", bufs=2, space="PSUM") as p2_psum:
                    pending_fin = []
                    ph3q = []

                    def make_ph3_thunks(c):
                        # output projection for chunk c as 32 independent
                        # 4-matmul groups: injected between attention tiles
                        # so the PE always has ready work while exp runs
                        thunks = []
                        for tl in range(tpc):
                            st = c * tpc + tl
                            o_acc = o_pool.tile([PT, MD], bf16, tag="oacc",
                                                name="o_acc")
                            for mc in range(n_mc):
                                def th(st=st, mc=mc, o_acc=o_acc):
                                    ps_o = p2_psum.tile(
                                        [PT, RH], f32, tag="po", bufs=2,
                                        name="ps_o")
                                    for rl in range(R):
                                        nc.tensor.matmul(
                                            ps_o[:],
                                            yT_sb[:, rl,
                                                  st * PT:(st + 1) * PT],
                                            wo_sb[:, rl,
                                                  mc * RH:(mc + 1) * RH],
                                            start=(rl == 0),
                                            stop=(rl == R - 1))
                                    nc.vector.tensor_copy(
                                        o_acc[:, mc * RH:(mc + 1) * RH],
                                        ps_o[:])
                                    if mc == n_mc - 1:
                                        nc.sync.dma_start(
                                            outp[:, st, :], o_acc[:])
                                thunks.append(th)
                        return thunks

                    def inject(n):
                        for _ in range(n):
                            if ph3q:
                                ph3q.pop(0)()

                    for c in range(n_sc):
                        T = (c + 1) * tpc
                        csl = slice(c * SC, (c + 1) * SC)
                        if phases >= 3 and c >= 1:
                            ph3q.extend(make_ph3_thunks(c - 1))
                        for j in range(R):
                            ps_y = p2_psum.tile([H, SC], f32, tag="py",
                                                bufs=2, name="ps_y")
                            ps_z = p2_psum.tile([1, SC], f32, tag="pz",
                                                bufs=1, name="ps_z")
                            rq = qT_sb[:, j, csl]
                            es = {}

                            def qk_exp(t, rq=rq, T=T, es=es):
                                ps_s = p2_psum.tile([PT, SC], f32,
                                                    tag="ps", bufs=3,
                                                    name="ps_s")
                                nc.tensor.matmul(
                                    ps_s[:],
                                    kT_sb[:, t * PT:(t + 1) * PT],
                                    rq, start=True, stop=True)
                                e_t = e_pool.tile([PT, SC], bf16,
                                                  tag="e", name="e_t")
                                nc.scalar.activation(
                                    e_t[:], ps_s[:],
                                    mybir.ActivationFunctionType.Exp,
                                    scale=SCALE)
                                dt = t - (T - tpc)
                                if dt >= 0:
                                    # zero the causally-masked region with
                                    # a precomputed 0/1 mask on DVE (low
                                    # latency, no Q7 launch overhead)
                                    nc.vector.tensor_mul(
                                        e_t[:], e_t[:], mask4[:, dt, :])
                                es[t] = e_t

                            # diagonal tiles first: their mask + exp
                            # latency hides under the previous head's tail
                            order = ([T - 1, T - 2, T - 3, T - 4]
                                     + list(range(T - 4)))
                            qk_exp(order[0])
                            while pending_fin:
                                pending_fin.pop(0)()
                            # z matmuls deferred two tiles behind AV:
                            # always-ready PE filler during exp waits
                            ny = 0
                            nz = 0
                            zq = []
                            for oi, t in enumerate(order):
                                if oi + 1 < T:
                                    qk_exp(order[oi + 1])
                                e_t = es.pop(t)
                                nc.tensor.matmul(
                                    ps_y[:], v_sb[:, t, :], e_t[:],
                                    start=(ny == 0), stop=(ny == T - 1))
                                ny += 1
                                zq.append(e_t)
                                inject(1)

                                def z_pair():
                                    # pre-sum a pair of e tiles on DVE so
                                    # half as many ones-matmuls occupy PE
                                    nonlocal nz
                                    e0 = zq.pop(0)
                                    e1 = zq.pop(0)
                                    ez = e_pool.tile([PT, SC], bf16,
                                                     tag="ez", bufs=3,
                                                     name="ez")
                                    nc.vector.tensor_add(
                                        ez[:], e0[:], e1[:])
                                    nc.tensor.matmul(
                                        ps_z[0:1, :], ones_b[:, 0:1],
                                        ez[:], start=(nz == 0),
                                        stop=(nz == T // 2 - 1))
                                    nz += 1

                                if oi > 0:
                                    while len(zq) > 3:
                                        z_pair()
                            while zq:
                                z_pair()

                            def finalize(c=c, j=j, ps_y=ps_y, ps_z=ps_z,
                                         csl=csl):
                                rz = z_pool.tile([1, SC], f32, tag="rz",
                                                 name="rz")
                                nc.vector.reciprocal_approx_fast(
                                    out=rz[:], in_=ps_z[0:1, :])
                                # broadcast 1/z across partitions on the
                                # (idle) Pool engine
                                b_sb = z_pool.tile([PT, SC], f32,
                                                   tag="bsb", name="b_sb")
                                nc.gpsimd.partition_broadcast(
                                    b_sb[:], rz[:])
                                nc.vector.tensor_mul(
                                    yT_sb[:, j, csl], ps_y[:], b_sb[:])

                            pending_fin.append(finalize)
                    while pending_fin:
                        pending_fin.pop(0)()
                    if phases >= 3:
                        ph3q.extend(make_ph3_thunks(n_sc - 1))
                        while ph3q:
                            ph3q.pop(0)()

            persist_ctx.__exit__(None, None, None)
    return nc


def _pack_pm(a):
    """[n_mt*128, C] -> [128, n_mt, C] partition-major, bf16."""
    n_mt = a.shape[0] // PT
    return np.ascontiguousarray(
        a.reshape(n_mt, PT, a.shape[1]).transpose(1, 0, 2)).astype(BFNP)


def shard_inputs(x, wq, wk, wv, wo, mask, sin, cos, s=S):
    del mask  # causality generated on device
    n_mt = MD // PT
    n_sc = s // SC
    xT = np.asarray(x, dtype=np.float32).reshape(s, MD).T  # [MD, s]
    # -> [128, n_mt, n_sc, SC] -> chunk-major [128, n_sc, n_mt*SC]
    x4 = xT.reshape(n_mt, PT, n_sc, SC).transpose(1, 2, 0, 3)
    xcp = np.ascontiguousarray(
        x4.reshape(PT, n_sc, n_mt * SC)).astype(BFNP)
    cosT = np.ascontiguousarray(
        np.asarray(cos, dtype=np.float32).T).astype(BFNP)
    sign = np.concatenate(
        [-np.ones((H // 2, 1)), np.ones((H // 2, 1))]).astype(np.float32)
    sinTs = np.ascontiguousarray(
        np.asarray(sin, dtype=np.float32).T * sign).astype(BFNP)
    wo = np.asarray(wo, dtype=np.float32)
    wq = np.asarray(wq, dtype=np.float32)
    wk = np.asarray(wk, dtype=np.float32)
    wv = np.asarray(wv, dtype=np.float32)
    in_maps = []
    for c in range(NCORES):
        in_maps.append({
            "xc": xcp,
            "wq": _pack_pm(np.ascontiguousarray(
                wq[:, :, c, :].reshape(MD, RH))),
            "wk": _pack_pm(np.ascontiguousarray(wk[:, c, :])),
            "wv": _pack_pm(np.ascontiguousarray(wv[:, c, :])),
            "wo": _pack_pm(np.ascontiguousarray(
                wo[:, c, :, :].reshape(RH, MD))),
            "cosT": cosT,
            "sinT": sinTs,
        })
    return in_maps


def unpack_out(outp_arr, s=S):
    """[128, s/128, MD] bf16 -> [s, MD] f32."""
    return np.ascontiguousarray(
        np.asarray(outp_arr).astype(np.float32).reshape(
            PT, s // PT, MD).transpose(1, 0, 2).reshape(s, MD))


_NC_CACHE = {}


def kernel(x, wq, wk, wv, wo, mask, sin, cos):
    s = x.shape[1]
    if s not in _NC_CACHE:
        _NC_CACHE[s] = build_bass(s)
    nc = _NC_CACHE[s]
    in_maps = shard_inputs(x, wq, wk, wv, wo, mask, sin, cos, s=s)
    res = run_bass_kernel_spmd(nc, in_maps, list(range(NCORES)))
    out = unpack_out(res.results[0]["outp"], s)
    for c in range(1, NCORES):
        out = out + unpack_out(res.results[c]["outp"], s)
    return out.reshape(1, s, MD).astype(np.float32)


# revision 4
# speedup vs baseline: 1.1088x; 1.0373x over previous
"""Trainium2 Bass kernel v4 for GQA attention (nn_Attention_50053548868012).

Deltas vs v3 (driven by phase ablation: p1=235us, p2=+185us, p3=+135us,
sum == total, i.e. the between-chunk ph3 interleave filled nothing
because the PE queue is strict FIFO):
 - phase 3 matmul groups are injected at TILE granularity inside the
   attention loops, so the PE always has ready work queued between a
   QK matmul and the AV matmul that waits on its exp.
 - QK/exp/AV restructured from pair (two-bank [128,1024]) to single-tile
   [128,512] granularity: shorter exp latency quantum (720ns vs 1147ns)
   in the QK->exp->AV chain, and ps_s fits 3 single banks (freeing one
   bank so ps_z gets its own and ph3's ps_o keeps 2).
 - phase-1 DMA ordering: first x sub-slab issued before the wq halves
   and x goes on the scalar HWDGE ring (weights on sync ring) so the
   first matmul group isn't queued behind 6MB of weights.
 - phase-1 spills split ACT/DVE to halve the chunk-boundary PSUM-bank
   serialization.
"""

import numpy as np
import ml_dtypes

import concourse.bass as bass
import concourse.tile as tile
from concourse import bacc, mybir
from concourse.bass_utils import run_bass_kernel_spmd
from concourse.masks import make_identity

NCORES = 8
S = 2048
MD = 4096
H = 128
R = 4
KV = 8
PT = 128          # partition tile
SC = 512          # free-dim chunk
RH = R * H        # 512
SCALE = float(H) ** -0.5
NEG = -30000.0

f32 = mybir.dt.float32
bf16 = mybir.dt.bfloat16
BFNP = ml_dtypes.bfloat16


def build_bass(s=S, collective=True, phases=3, reps=1):
    nc = _emit(s, collective, phases, reps)
    nc.compile()
    return nc


def _emit(s, collective, phases, reps=1):
    assert s % SC == 0
    n_sc = s // SC          # seq chunks (4)
    n_mt = MD // PT         # model-dim tiles (32)
    n_tt = s // PT          # seq tiles of 128 (16)
    tpc = SC // PT          # 128-tiles per chunk (4)
    hh = H // 2

    nc = bacc.Bacc("TRN2", target_bir_lowering=False, debug=False,
                   num_devices=NCORES)

    # x chunk-major: [128, chunk, m_tile * 512] so one chunk slab is a
    # single contiguous DMA
    xc = nc.dram_tensor("xc", [PT, n_sc, n_mt * SC], bf16,
                        kind="ExternalInput").ap()
    wq = nc.dram_tensor("wq", [PT, n_mt, RH], bf16, kind="ExternalInput").ap()
    wk = nc.dram_tensor("wk", [PT, n_mt, H], bf16, kind="ExternalInput").ap()
    wv = nc.dram_tensor("wv", [PT, n_mt, H], bf16, kind="ExternalInput").ap()
    wo = nc.dram_tensor("wo", [PT, R, MD], bf16, kind="ExternalInput").ap()
    cosT = nc.dram_tensor("cosT", [H, s], bf16, kind="ExternalInput").ap()
    sinT = nc.dram_tensor("sinT", [H, s], bf16, kind="ExternalInput").ap()
    outp = nc.dram_tensor("outp", [PT, n_tt, MD], bf16,
                          kind="ExternalOutput").ap()

    with tile.TileContext(nc) as tc:
      for _rep in range(reps):
        with tc.tile_pool(name="const", bufs=1) as const_pool:
            ones_f = const_pool.tile([PT, PT], f32)
            nc.gpsimd.memset(ones_f[:], 1.0)
            ones_b = const_pool.tile([PT, PT], bf16)
            nc.scalar.copy(ones_b[:], ones_f[:])
            ident = const_pool.tile([PT, PT], bf16)
            make_identity(nc, ident[:])
            cos_sb = const_pool.tile([H, s], bf16)
            nc.gpsimd.dma_start(cos_sb[:], cosT)
            sin_sb = const_pool.tile([H, s], bf16)
            nc.gpsimd.dma_start(sin_sb[:], sinT)
            # 0/1 causal masks for the 4 diagonal-block offsets:
            # mask4[p, dt, q] = 1 where q >= p + 128*dt else 0
            mask4 = const_pool.tile([PT, tpc, SC], bf16)
            nc.gpsimd.memset(mask4[:], 1.0)
            nc.gpsimd.affine_select(
                out=mask4[:], in_=mask4[:],
                compare_op=mybir.AluOpType.is_ge, fill=0.0, base=0,
                pattern=[[-PT, tpc], [1, SC]], channel_multiplier=-1)

            persist_ctx = tc.tile_pool(name="persist", bufs=1)
            persist = persist_ctx.__enter__()
            qT_sb = persist.tile([H, R, s], bf16)
            kT_sb = persist.tile([H, s], bf16)
            v_sb = persist.tile([PT, n_tt, H], bf16)
            yT_sb = persist.tile([H, R, s], bf16)

            # ---------- Phase 1: projections (chunk-outer) + inline RoPE --
            with tc.tile_pool(name="w1", bufs=1) as w_pool, \
                 tc.tile_pool(name="xslab", bufs=2) as x_pool, \
                 tc.tile_pool(name="acc1", bufs=2) as acc_pool, \
                 tc.tile_pool(name="rope", bufs=2) as rope_pool, \
                 tc.tile_pool(name="p1ps", bufs=1, space="PSUM") as p1_psum, \
                 tc.tile_pool(name="tpps", bufs=2, space="PSUM") as tp_psum:
                # k/v weights first so the PE can start early; wq streams
                # under the first k/v matmuls.  x slabs go on the scalar
                # HWDGE ring so they aren't queued behind the weights.
                wk_b = w_pool.tile([PT, n_mt, H], bf16)
                nc.sync.dma_start(wk_b[:], wk)
                wv_b = w_pool.tile([PT, n_mt, H], bf16)
                nc.sync.dma_start(wv_b[:], wv)
                wq_b = w_pool.tile([PT, n_mt, RH], bf16)
                for q4 in range(4):
                    msl = slice(q4 * (n_mt // 4), (q4 + 1) * (n_mt // 4))
                    nc.sync.dma_start(wq_b[:, msl, :], wq[:, msl, :])
                pending_rope = []

                for sc_i in range(n_sc):
                    ssl = slice(sc_i * SC, (sc_i + 1) * SC)
                    xs = x_pool.tile([PT, n_mt, SC], bf16, tag="xs",
                                     name="xs")
                    # split the 4MB chunk load so matmuls start after 1MB
                    for q4 in range(4):
                        msl = slice(q4 * (n_mt // 4), (q4 + 1) * (n_mt // 4))
                        nc.sync.dma_start(xs[:, msl, :], xc[:, sc_i, :]
                                          .rearrange("p (m c) -> p m c",
                                                     m=n_mt)[:, msl, :])
                    ps6 = [p1_psum.tile([PT, SC], f32, tag=f"pa{u}",
                                        name=f"ps6_{u}")
                           for u in range(R + 2)]
                    for ml in range(n_mt):
                        rx = xs[:, ml, :]
                        st = ml == 0
                        sp = ml == n_mt - 1
                        nc.tensor.matmul(
                            ps6[R][:], wk_b[:, ml, :], rx, start=st, stop=sp)
                        nc.tensor.matmul(
                            ps6[R + 1][:], wv_b[:, ml, :], rx,
                            start=st, stop=sp)
                        for j in range(R):
                            nc.tensor.matmul(
                                ps6[j][:], wq_b[:, ml, j * H:(j + 1) * H],
                                rx, start=st, stop=sp)
                    # single spill PSUM->SBUF on ACT (idle in phase 1)
                    qacc = acc_pool.tile([H, R, SC], bf16, tag="qa",
                                         name="qacc")
                    kacc = acc_pool.tile([H, SC], bf16, tag="ka",
                                         name="kacc")
                    vacc = acc_pool.tile([H, SC], bf16, tag="va",
                                         name="vacc")
                    # split spills ACT/DVE so the 6 PSUM banks free in
                    # ~half the serial time at the chunk boundary
                    nc.scalar.copy(qacc[:, 0, :], ps6[0][:])
                    nc.vector.tensor_copy(qacc[:, 1, :], ps6[1][:])
                    nc.scalar.copy(qacc[:, 2, :], ps6[2][:])
                    nc.vector.tensor_copy(qacc[:, 3, :], ps6[3][:])
                    nc.scalar.copy(kacc[:], ps6[R][:])
                    nc.vector.tensor_copy(vacc[:], ps6[R + 1][:])

                    # rope swap-halves via Pool SWDGE (overlapped); the DVE
                    # rope math is DEFERRED one chunk so it queues behind
                    # the next chunk's matmul stream
                    qsw = rope_pool.tile([H, R, SC], bf16, tag="qsw",
                                         name="qsw")
                    nc.gpsimd.dma_start(qsw[0:hh, :, :], qacc[hh:H, :, :])
                    nc.gpsimd.dma_start(qsw[hh:H, :, :], qacc[0:hh, :, :])
                    ksw = rope_pool.tile([H, SC], bf16, tag="ksw",
                                         name="ksw")
                    nc.gpsimd.dma_start(ksw[0:hh, :], kacc[hh:H, :])
                    nc.gpsimd.dma_start(ksw[hh:H, :], kacc[0:hh, :])

                    def rope_math(sc_i=sc_i, ssl=ssl, qsw=qsw, ksw=ksw,
                                  qacc=qacc, kacc=kacc, vacc=vacc):
                        for j in range(R):
                            nc.vector.tensor_mul(
                                qsw[:, j, :], qsw[:, j, :], sin_sb[:, ssl])
                            nc.vector.tensor_mul(
                                qT_sb[:, j, ssl], qacc[:, j, :],
                                cos_sb[:, ssl])
                            nc.vector.tensor_add(
                                qT_sb[:, j, ssl], qT_sb[:, j, ssl],
                                qsw[:, j, :])
                        nc.vector.tensor_mul(ksw[:], ksw[:], sin_sb[:, ssl])
                        nc.vector.tensor_mul(
                            kT_sb[:, ssl], kacc[:], cos_sb[:, ssl])
                        nc.vector.tensor_add(
                            kT_sb[:, ssl], kT_sb[:, ssl], ksw[:])
                        for tl in range(tpc):
                            tt = sc_i * tpc + tl
                            ps_t = tp_psum.tile([PT, PT], bf16, tag="tp",
                                                name="ps_t")
                            nc.tensor.transpose(
                                ps_t[:], vacc[:, tl * PT:(tl + 1) * PT],
                                ident[:])
                            nc.scalar.copy(v_sb[:, tt, :], ps_t[:])

                    pending_rope.append(rope_math)
                    if len(pending_rope) > 1:
                        pending_rope.pop(0)()
                while pending_rope:
                    pending_rope.pop(0)()

            if phases >= 2:
              # wo pool spans phases 2+3 (prefetch overlaps attention)
              with tc.tile_pool(name="w3", bufs=1) as w3_pool:
                wo_sb = w3_pool.tile([PT, R, MD], bf16)
                for rl in range(R):
                    nc.sync.dma_start(wo_sb[:, rl, :], wo[:, rl, :])

                # ------- Phase 2: attention (+ tile-level phase 3) -------
                n_mc = MD // RH
                with tc.tile_pool(name="epool", bufs=8) as e_pool, \
                     tc.tile_pool(name="zpool", bufs=2) as z_pool, \
                     tc.tile_pool(name="osb", bufs=3) as o_pool, \
                     tc.tile_pool(name="p2ps", bufs=2, space="PSUM") as p2_psum:
                    pending_fin = []
                    ph3q = []

                    def make_ph3_thunks(c):
                        # output projection for chunk c as 32 independent
                        # 4-matmul groups: injected between attention tiles
                        # so the PE always has ready work while exp runs
                        thunks = []
                        for tl in range(tpc):
                            st = c * tpc + tl
                            o_acc = o_pool.tile([PT, MD], bf16, tag="oacc",
                                                name="o_acc")
                            for mc in range(n_mc):
                                def th(st=st, mc=mc, o_acc=o_acc):
                                    ps_o = p2_psum.tile(
                                        [PT, RH], f32, tag="po", bufs=2,
                                        name="ps_o")
                                    for rl in range(R):
                                        nc.tensor.matmul(
                                            ps_o[:],
                                            yT_sb[:, rl,
                                                  st * PT:(st + 1) * PT],
                                            wo_sb[:, rl,
                                                  mc * RH:(mc + 1) * RH],
                                            start=(rl == 0),
                                            stop=(rl == R - 1))
                                    nc.vector.tensor_copy(
                                        o_acc[:, mc * RH:(mc + 1) * RH],
                                        ps_o[:])
                                    if mc == n_mc - 1:
                                        nc.sync.dma_start(
                                            outp[:, st, :], o_acc[:])
                                thunks.append(th)
                        return thunks

                    def inject(n):
                        for _ in range(n):
                            if ph3q:
                                ph3q.pop(0)()

                    for c in range(n_sc):
                        T = (c + 1) * tpc
                        csl = slice(c * SC, (c + 1) * SC)
                        if phases >= 3 and c >= 1:
                            ph3q.extend(make_ph3_thunks(c - 1))
                        for j in range(R):
                            ps_y = p2_psum.tile([H, SC], f32, tag="py",
                                                bufs=2, name="ps_y")
                            ps_z = p2_psum.tile([1, SC], f32, tag="pz",
                                                bufs=1, name="ps_z")
                            rq = qT_sb[:, j, csl]
                            es = {}

                            def qk_exp(t, rq=rq, T=T, es=es):
                                ps_s = p2_psum.tile([PT, SC], f32,
                                                    tag="ps", bufs=3,
                                                    name="ps_s")
                                nc.tensor.matmul(
                                    ps_s[:],
                                    kT_sb[:, t * PT:(t + 1) * PT],
                                    rq, start=True, stop=True)
                                e_t = e_pool.tile([PT, SC], bf16,
                                                  tag="e", name="e_t")
                                nc.scalar.activation(
                                    e_t[:], ps_s[:],
                                    mybir.ActivationFunctionType.Exp,
                                    scale=SCALE)
                                dt = t - (T - tpc)
                                if dt >= 0:
                                    # zero the causally-masked region on
                                    # Pool (DVE is busy with z-adds and
                                    # spill copies; diag-first order hides
                                    # the Q7 launch latency)
                                    nc.gpsimd.affine_select(
                                        out=e_t[:], in_=e_t[:],
                                        compare_op=mybir.AluOpType.is_ge,
                                        fill=0.0, base=-PT * dt,
                                        pattern=[[1, SC]],
                                        channel_multiplier=-1)
                                es[t] = e_t

                            # diagonal tiles first: their mask + exp
                            # latency hides under the previous head's tail
                            order = ([T - 1, T - 2, T - 3, T - 4]
                                     + list(range(T - 4)))
                            qk_exp(order[0])
                            while pending_fin:
                                pending_fin.pop(0)()
                            # z matmuls deferred two tiles behind AV:
                            # always-ready PE filler during exp waits
                            ny = 0
                            nz = 0
                            zq = []
                            for oi, t in enumerate(order):
                                if oi + 1 < T:
                                    qk_exp(order[oi + 1])
                                e_t = es.pop(t)
                                nc.tensor.matmul(
                                    ps_y[:], v_sb[:, t, :], e_t[:],
                                    start=(ny == 0), stop=(ny == T - 1))
                                ny += 1
                                zq.append(e_t)
                                inject(1)

                                def z_quad():
                                    # pre-sum four e tiles on DVE so a
                                    # quarter as many ones-matmuls occupy
                                    # PE (T is always a multiple of 4)
                                    nonlocal nz
                                    e0 = zq.pop(0)
                                    e1 = zq.pop(0)
                                    e2 = zq.pop(0)
                                    e3 = zq.pop(0)
                                    ez = e_pool.tile([PT, SC], bf16,
                                                     tag="ez", bufs=3,
                                                     name="ez")
                                    nc.vector.tensor_add(
                                        ez[:], e0[:], e1[:])
                                    nc.vector.tensor_add(
                                        ez[:], ez[:], e2[:])
                                    nc.vector.tensor_add(
                                        ez[:], ez[:], e3[:])
                                    nc.tensor.matmul(
                                        ps_z[0:1, :], ones_b[:, 0:1],
                                        ez[:], start=(nz == 0),
                                        stop=(nz == T // 4 - 1))
                                    nz += 1

                                if oi > 0:
                                    while len(zq) > 5:
                                        z_quad()
                            while zq:
                                z_quad()

                            def finalize(c=c, j=j, ps_y=ps_y, ps_z=ps_z,
                                         csl=csl):
                                rz = z_pool.tile([1, SC], f32, tag="rz",
                                                 name="rz")
                                nc.vector.reciprocal_approx_fast(
                                    out=rz[:], in_=ps_z[0:1, :])
                                # broadcast 1/z across partitions on the
                                # (idle) Pool engine
                                b_sb = z_pool.tile([PT, SC], f32,
                                                   tag="bsb", name="b_sb")
                                nc.gpsimd.partition_broadcast(
                                    b_sb[:], rz[:])
                                nc.vector.tensor_mul(
                                    yT_sb[:, j, csl], ps_y[:], b_sb[:])

                            pending_fin.append(finalize)
                    while pending_fin:
                        pending_fin.pop(0)()
                    if phases >= 3:
                        ph3q.extend(make_ph3_thunks(n_sc - 1))
                        while ph3q:
                            ph3q.pop(0)()

            persist_ctx.__exit__(None, None, None)
    return nc


def _pack_pm(a):
    """[n_mt*128, C] -> [128, n_mt, C] partition-major, bf16."""
    n_mt = a.shape[0] // PT
    return np.ascontiguousarray(
        a.reshape(n_mt, PT, a.shape[1]).transpose(1, 0, 2)).astype(BFNP)


def shard_inputs(x, wq, wk, wv, wo, mask, sin, cos, s=S):
    del mask  # causality generated on device
    n_mt = MD // PT
    n_sc = s // SC
    xT = np.asarray(x, dtype=np.float32).reshape(s, MD).T  # [MD, s]
    # -> [128, n_mt, n_sc, SC] -> chunk-major [128, n_sc, n_mt*SC]
    x4 = xT.reshape(n_mt, PT, n_sc, SC).transpose(1, 2, 0, 3)
    xcp = np.ascontiguousarray(
        x4.reshape(PT, n_sc, n_mt * SC)).astype(BFNP)
    cosT = np.ascontiguousarray(
        np.asarray(cos, dtype=np.float32).T).astype(BFNP)
    sign = np.concatenate(
        [-np.ones((H // 2, 1)), np.ones((H // 2, 1))]).astype(np.float32)
    sinTs = np.ascontiguousarray(
        np.asarray(sin, dtype=np.float32).T * sign).astype(BFNP)
    wo = np.asarray(wo, dtype=np.float32)
    wq = np.asarray(wq, dtype=np.float32)
    wk = np.asarray(wk, dtype=np.float32)
    wv = np.asarray(wv, dtype=np.float32)
    in_maps = []
    for c in range(NCORES):
        in_maps.append({
            "xc": xcp,
            "wq": _pack_pm(np.ascontiguousarray(
                wq[:, :, c, :].reshape(MD, RH))),
            "wk": _pack_pm(np.ascontiguousarray(wk[:, c, :])),
            "wv": _pack_pm(np.ascontiguousarray(wv[:, c, :])),
            "wo": _pack_pm(np.ascontiguousarray(
                wo[:, c, :, :].reshape(RH, MD))),
            "cosT": cosT,
            "sinT": sinTs,
        })
    return in_maps


def unpack_out(outp_arr, s=S):
    """[128, s/128, MD] bf16 -> [s, MD] f32."""
    return np.ascontiguousarray(
        np.asarray(outp_arr).astype(np.float32).reshape(
            PT, s // PT, MD).transpose(1, 0, 2).reshape(s, MD))


_NC_CACHE = {}


def kernel(x, wq, wk, wv, wo, mask, sin, cos):
    s = x.shape[1]
    if s not in _NC_CACHE:
        _NC_CACHE[s] = build_bass(s)
    nc = _NC_CACHE[s]
    in_maps = shard_inputs(x, wq, wk, wv, wo, mask, sin, cos, s=s)
    res = run_bass_kernel_spmd(nc, in_maps, list(range(NCORES)))
    out = unpack_out(res.results[0]["outp"], s)
    for c in range(1, NCORES):
        out = out + unpack_out(res.results[c]["outp"], s)
    return out.reshape(1, s, MD).astype(np.float32)


# revision 5
# speedup vs baseline: 1.1168x; 1.0072x over previous
"""Trainium2 Bass kernel v4 for GQA attention (nn_Attention_50053548868012).

Deltas vs v3 (driven by phase ablation: p1=235us, p2=+185us, p3=+135us,
sum == total, i.e. the between-chunk ph3 interleave filled nothing
because the PE queue is strict FIFO):
 - phase 3 matmul groups are injected at TILE granularity inside the
   attention loops, so the PE always has ready work queued between a
   QK matmul and the AV matmul that waits on its exp.
 - QK/exp/AV restructured from pair (two-bank [128,1024]) to single-tile
   [128,512] granularity: shorter exp latency quantum (720ns vs 1147ns)
   in the QK->exp->AV chain, and ps_s fits 3 single banks (freeing one
   bank so ps_z gets its own and ph3's ps_o keeps 2).
 - phase-1 DMA ordering: first x sub-slab issued before the wq halves
   and x goes on the scalar HWDGE ring (weights on sync ring) so the
   first matmul group isn't queued behind 6MB of weights.
 - phase-1 spills split ACT/DVE to halve the chunk-boundary PSUM-bank
   serialization.
"""

import numpy as np
import ml_dtypes

import concourse.bass as bass
import concourse.tile as tile
from concourse import bacc, mybir
from concourse.bass_utils import run_bass_kernel_spmd
from concourse.masks import make_identity

NCORES = 8
S = 2048
MD = 4096
H = 128
R = 4
KV = 8
PT = 128          # partition tile
SC = 512          # free-dim chunk
RH = R * H        # 512
SCALE = float(H) ** -0.5
NEG = -30000.0

f32 = mybir.dt.float32
bf16 = mybir.dt.bfloat16
BFNP = ml_dtypes.bfloat16


def build_bass(s=S, collective=True, phases=3, reps=1):
    nc = _emit(s, collective, phases, reps)
    nc.compile()
    return nc


def _emit(s, collective, phases, reps=1):
    assert s % SC == 0
    n_sc = s // SC          # seq chunks (4)
    n_mt = MD // PT         # model-dim tiles (32)
    n_tt = s // PT          # seq tiles of 128 (16)
    tpc = SC // PT          # 128-tiles per chunk (4)
    hh = H // 2

    nc = bacc.Bacc("TRN2", target_bir_lowering=False, debug=False,
                   num_devices=NCORES)

    # x chunk-major: [128, chunk, m_tile * 512] so one chunk slab is a
    # single contiguous DMA
    xc = nc.dram_tensor("xc", [PT, n_sc, n_mt * SC], bf16,
                        kind="ExternalInput").ap()
    wq = nc.dram_tensor("wq", [PT, n_mt, RH], bf16, kind="ExternalInput").ap()
    wk = nc.dram_tensor("wk", [PT, n_mt, H], bf16, kind="ExternalInput").ap()
    wv = nc.dram_tensor("wv", [PT, n_mt, H], bf16, kind="ExternalInput").ap()
    wo = nc.dram_tensor("wo", [PT, R, MD], bf16, kind="ExternalInput").ap()
    cosT = nc.dram_tensor("cosT", [H, s], bf16, kind="ExternalInput").ap()
    sinT = nc.dram_tensor("sinT", [H, s], bf16, kind="ExternalInput").ap()
    outp = nc.dram_tensor("outp", [PT, n_tt, MD], bf16,
                          kind="ExternalOutput").ap()

    with tile.TileContext(nc) as tc:
      for _rep in range(reps):
        with tc.tile_pool(name="const", bufs=1) as const_pool:
            ones_f = const_pool.tile([PT, PT], f32)
            nc.gpsimd.memset(ones_f[:], 1.0)
            ones_b = const_pool.tile([PT, PT], bf16)
            nc.scalar.copy(ones_b[:], ones_f[:])
            ident = const_pool.tile([PT, PT], bf16)
            make_identity(nc, ident[:])
            cos_sb = const_pool.tile([H, s], bf16)
            nc.gpsimd.dma_start(cos_sb[:], cosT)
            sin_sb = const_pool.tile([H, s], bf16)
            nc.gpsimd.dma_start(sin_sb[:], sinT)
            # 0/1 causal masks for the 4 diagonal-block offsets:
            # mask4[p, dt, q] = 1 where q >= p + 128*dt else 0
            mask4 = const_pool.tile([PT, tpc, SC], bf16)
            nc.gpsimd.memset(mask4[:], 1.0)
            nc.gpsimd.affine_select(
                out=mask4[:], in_=mask4[:],
                compare_op=mybir.AluOpType.is_ge, fill=0.0, base=0,
                pattern=[[-PT, tpc], [1, SC]], channel_multiplier=-1)

            persist_ctx = tc.tile_pool(name="persist", bufs=1)
            persist = persist_ctx.__enter__()
            qT_sb = persist.tile([H, R, s], bf16)
            kT_sb = persist.tile([H, s], bf16)
            v_sb = persist.tile([PT, n_tt, H], bf16)
            yT_sb = persist.tile([H, R, s], bf16)

            # ---------- Phase 1: projections (chunk-outer) + inline RoPE --
            with tc.tile_pool(name="w1", bufs=1) as w_pool, \
                 tc.tile_pool(name="xslab", bufs=2) as x_pool, \
                 tc.tile_pool(name="acc1", bufs=2) as acc_pool, \
                 tc.tile_pool(name="rope", bufs=2) as rope_pool, \
                 tc.tile_pool(name="p1ps", bufs=1, space="PSUM") as p1_psum, \
                 tc.tile_pool(name="tpps", bufs=2, space="PSUM") as tp_psum:
                # k/v weights first so the PE can start early; wq streams
                # under the first k/v matmuls.  x slabs go on the scalar
                # HWDGE ring so they aren't queued behind the weights.
                wk_b = w_pool.tile([PT, n_mt, H], bf16)
                nc.sync.dma_start(wk_b[:], wk)
                wv_b = w_pool.tile([PT, n_mt, H], bf16)
                nc.sync.dma_start(wv_b[:], wv)
                wq_b = w_pool.tile([PT, n_mt, RH], bf16)
                for q4 in range(4):
                    msl = slice(q4 * (n_mt // 4), (q4 + 1) * (n_mt // 4))
                    nc.sync.dma_start(wq_b[:, msl, :], wq[:, msl, :])
                pending_rope = []

                for sc_i in range(n_sc):
                    ssl = slice(sc_i * SC, (sc_i + 1) * SC)
                    xs = x_pool.tile([PT, n_mt, SC], bf16, tag="xs",
                                     name="xs")
                    # split the 4MB chunk load so matmuls start after 1MB
                    for q4 in range(4):
                        msl = slice(q4 * (n_mt // 4), (q4 + 1) * (n_mt // 4))
                        nc.sync.dma_start(xs[:, msl, :], xc[:, sc_i, :]
                                          .rearrange("p (m c) -> p m c",
                                                     m=n_mt)[:, msl, :])
                    ps6 = [p1_psum.tile([PT, SC], f32, tag=f"pa{u}",
                                        name=f"ps6_{u}")
                           for u in range(R + 2)]
                    for ml in range(n_mt):
                        rx = xs[:, ml, :]
                        st = ml == 0
                        sp = ml == n_mt - 1
                        nc.tensor.matmul(
                            ps6[R][:], wk_b[:, ml, :], rx, start=st, stop=sp)
                        nc.tensor.matmul(
                            ps6[R + 1][:], wv_b[:, ml, :], rx,
                            start=st, stop=sp)
                        for j in range(R):
                            nc.tensor.matmul(
                                ps6[j][:], wq_b[:, ml, j * H:(j + 1) * H],
                                rx, start=st, stop=sp)
                    # single spill PSUM->SBUF on ACT (idle in phase 1)
                    qacc = acc_pool.tile([H, R, SC], bf16, tag="qa",
                                         name="qacc")
                    kacc = acc_pool.tile([H, SC], bf16, tag="ka",
                                         name="kacc")
                    vacc = acc_pool.tile([H, SC], bf16, tag="va",
                                         name="vacc")
                    # split spills ACT/DVE so the 6 PSUM banks free in
                    # ~half the serial time at the chunk boundary
                    nc.scalar.copy(qacc[:, 0, :], ps6[0][:])
                    nc.vector.tensor_copy(qacc[:, 1, :], ps6[1][:])
                    nc.scalar.copy(qacc[:, 2, :], ps6[2][:])
                    nc.vector.tensor_copy(qacc[:, 3, :], ps6[3][:])
                    nc.scalar.copy(kacc[:], ps6[R][:])
                    nc.vector.tensor_copy(vacc[:], ps6[R + 1][:])

                    # rope swap-halves via Pool SWDGE (overlapped); the DVE
                    # rope math is DEFERRED one chunk so it queues behind
                    # the next chunk's matmul stream
                    qsw = rope_pool.tile([H, R, SC], bf16, tag="qsw",
                                         name="qsw")
                    nc.gpsimd.dma_start(qsw[0:hh, :, :], qacc[hh:H, :, :])
                    nc.gpsimd.dma_start(qsw[hh:H, :, :], qacc[0:hh, :, :])
                    ksw = rope_pool.tile([H, SC], bf16, tag="ksw",
                                         name="ksw")
                    nc.gpsimd.dma_start(ksw[0:hh, :], kacc[hh:H, :])
                    nc.gpsimd.dma_start(ksw[hh:H, :], kacc[0:hh, :])

                    def rope_math(sc_i=sc_i, ssl=ssl, qsw=qsw, ksw=ksw,
                                  qacc=qacc, kacc=kacc, vacc=vacc):
                        for j in range(R):
                            nc.vector.tensor_mul(
                                qsw[:, j, :], qsw[:, j, :], sin_sb[:, ssl])
                            nc.vector.tensor_mul(
                                qT_sb[:, j, ssl], qacc[:, j, :],
                                cos_sb[:, ssl])
                            nc.vector.tensor_add(
                                qT_sb[:, j, ssl], qT_sb[:, j, ssl],
                                qsw[:, j, :])
                        nc.vector.tensor_mul(ksw[:], ksw[:], sin_sb[:, ssl])
                        nc.vector.tensor_mul(
                            kT_sb[:, ssl], kacc[:], cos_sb[:, ssl])
                        nc.vector.tensor_add(
                            kT_sb[:, ssl], kT_sb[:, ssl], ksw[:])
                        for tl in range(tpc):
                            tt = sc_i * tpc + tl
                            ps_t = tp_psum.tile([PT, PT], bf16, tag="tp",
                                                name="ps_t")
                            nc.tensor.transpose(
                                ps_t[:], vacc[:, tl * PT:(tl + 1) * PT],
                                ident[:])
                            nc.scalar.copy(v_sb[:, tt, :], ps_t[:])

                    pending_rope.append(rope_math)
                    if len(pending_rope) > 1:
                        pending_rope.pop(0)()
                while pending_rope:
                    pending_rope.pop(0)()

            if phases >= 2:
              # wo pool spans phases 2+3 (prefetch overlaps attention)
              with tc.tile_pool(name="w3", bufs=1) as w3_pool:
                wo_sb = w3_pool.tile([PT, R, MD], bf16)
                for rl in range(R):
                    nc.sync.dma_start(wo_sb[:, rl, :], wo[:, rl, :])

                # ------- Phase 2: attention (+ tile-level phase 3) -------
                n_mc = MD // RH
                with tc.tile_pool(name="epool", bufs=8) as e_pool, \
                     tc.tile_pool(name="zpool", bufs=2) as z_pool, \
                     tc.tile_pool(name="osb", bufs=3) as o_pool, \
                     tc.tile_pool(name="p2ps", bufs=2, space="PSUM") as p2_psum:
                    pending_fin = []
                    ph3q = []

                    def make_ph3_thunks(c):
                        # output projection for chunk c as 32 independent
                        # 4-matmul groups: injected between attention tiles
                        # so the PE always has ready work while exp runs
                        thunks = []
                        for tl in range(tpc):
                            st = c * tpc + tl
                            o_acc = o_pool.tile([PT, MD], bf16, tag="oacc",
                                                name="o_acc")
                            for mc in range(n_mc):
                                def th(st=st, mc=mc, o_acc=o_acc):
                                    ps_o = p2_psum.tile(
                                        [PT, RH], f32, tag="po", bufs=2,
                                        name="ps_o")
                                    for rl in range(R):
                                        nc.tensor.matmul(
                                            ps_o[:],
                                            yT_sb[:, rl,
                                                  st * PT:(st + 1) * PT],
                                            wo_sb[:, rl,
                                                  mc * RH:(mc + 1) * RH],
                                            start=(rl == 0),
                                            stop=(rl == R - 1))
                                    nc.vector.tensor_copy(
                                        o_acc[:, mc * RH:(mc + 1) * RH],
                                        ps_o[:])
                                    if mc == n_mc - 1:
                                        nc.sync.dma_start(
                                            outp[:, st, :], o_acc[:])
                                thunks.append(th)
                        return thunks

                    def inject(n):
                        for _ in range(n):
                            if ph3q:
                                ph3q.pop(0)()

                    for c in range(n_sc):
                        T = (c + 1) * tpc
                        csl = slice(c * SC, (c + 1) * SC)
                        if phases >= 3 and c >= 1:
                            ph3q.extend(make_ph3_thunks(c - 1))
                        tile_ix = [0]
                        n_ct = R * T

                        def paced_inject(tile_ix=tile_ix, n_ct=n_ct):
                            # spread the chunk's <=32 ph3 thunks over all
                            # its R*T attention tiles instead of
                            # front-loading them
                            want = min(32, n_ct) * (tile_ix[0] + 1) // n_ct
                            done = min(32, n_ct) * tile_ix[0] // n_ct
                            tile_ix[0] += 1
                            inject(want - done)

                        for j in range(R):
                            ps_y = p2_psum.tile([H, SC], f32, tag="py",
                                                bufs=2, name="ps_y")
                            ps_z = p2_psum.tile([1, SC], f32, tag="pz",
                                                bufs=1, name="ps_z")
                            rq = qT_sb[:, j, csl]
                            es = {}

                            def qk_exp(t, rq=rq, T=T, es=es):
                                ps_s = p2_psum.tile([PT, SC], f32,
                                                    tag="ps", bufs=3,
                                                    name="ps_s")
                                nc.tensor.matmul(
                                    ps_s[:],
                                    kT_sb[:, t * PT:(t + 1) * PT],
                                    rq, start=True, stop=True)
                                e_t = e_pool.tile([PT, SC], bf16,
                                                  tag="e", name="e_t")
                                nc.scalar.activation(
                                    e_t[:], ps_s[:],
                                    mybir.ActivationFunctionType.Exp,
                                    scale=SCALE)
                                dt = t - (T - tpc)
                                if dt >= 0:
                                    # zero the causally-masked region on
                                    # Pool (DVE is busy with z-adds and
                                    # spill copies; diag-first order hides
                                    # the Q7 launch latency)
                                    nc.gpsimd.affine_select(
                                        out=e_t[:], in_=e_t[:],
                                        compare_op=mybir.AluOpType.is_ge,
                                        fill=0.0, base=-PT * dt,
                                        pattern=[[1, SC]],
                                        channel_multiplier=-1)
                                es[t] = e_t

                            # diagonal tiles first: their mask + exp
                            # latency hides under the previous head's tail
                            order = ([T - 1, T - 2, T - 3, T - 4]
                                     + list(range(T - 4)))
                            qk_exp(order[0])
                            while pending_fin:
                                pending_fin.pop(0)()
                            # z matmuls deferred two tiles behind AV:
                            # always-ready PE filler during exp waits
                            ny = 0
                            nz = 0
                            zq = []
                            for oi, t in enumerate(order):
                                if oi + 1 < T:
                                    qk_exp(order[oi + 1])
                                e_t = es.pop(t)
                                nc.tensor.matmul(
                                    ps_y[:], v_sb[:, t, :], e_t[:],
                                    start=(ny == 0), stop=(ny == T - 1))
                                ny += 1
                                zq.append(e_t)
                                paced_inject()

                                def z_quad():
                                    # pre-sum four e tiles on DVE so a
                                    # quarter as many ones-matmuls occupy
                                    # PE (T is always a multiple of 4)
                                    nonlocal nz
                                    e0 = zq.pop(0)
                                    e1 = zq.pop(0)
                                    e2 = zq.pop(0)
                                    e3 = zq.pop(0)
                                    ez = e_pool.tile([PT, SC], bf16,
                                                     tag="ez", bufs=3,
                                                     name="ez")
                                    nc.vector.tensor_add(
                                        ez[:], e0[:], e1[:])
                                    nc.vector.tensor_add(
                                        ez[:], ez[:], e2[:])
                                    nc.vector.tensor_add(
                                        ez[:], ez[:], e3[:])
                                    nc.tensor.matmul(
                                        ps_z[0:1, :], ones_b[:, 0:1],
                                        ez[:], start=(nz == 0),
                                        stop=(nz == T // 4 - 1))
                                    nz += 1

                                if oi > 0:
                                    while len(zq) > 5:
                                        z_quad()
                            while zq:
                                z_quad()

                            def finalize(c=c, j=j, ps_y=ps_y, ps_z=ps_z,
                                         csl=csl):
                                rz = z_pool.tile([1, SC], f32, tag="rz",
                                                 name="rz")
                                nc.vector.reciprocal_approx_fast(
                                    out=rz[:], in_=ps_z[0:1, :])
                                # broadcast 1/z across partitions on the
                                # (idle) Pool engine
                                b_sb = z_pool.tile([PT, SC], f32,
                                                   tag="bsb", name="b_sb")
                                nc.gpsimd.partition_broadcast(
                                    b_sb[:], rz[:])
                                nc.vector.tensor_mul(
                                    yT_sb[:, j, csl], ps_y[:], b_sb[:])

                            pending_fin.append(finalize)
                    while pending_fin:
                        pending_fin.pop(0)()
                    if phases >= 3:
                        ph3q.extend(make_ph3_thunks(n_sc - 1))
                        while ph3q:
                            ph3q.pop(0)()

            persist_ctx.__exit__(None, None, None)
    return nc


def _pack_pm(a):
    """[n_mt*128, C] -> [128, n_mt, C] partition-major, bf16."""
    n_mt = a.shape[0] // PT
    return np.ascontiguousarray(
        a.reshape(n_mt, PT, a.shape[1]).transpose(1, 0, 2)).astype(BFNP)


def shard_inputs(x, wq, wk, wv, wo, mask, sin, cos, s=S):
    del mask  # causality generated on device
    n_mt = MD // PT
    n_sc = s // SC
    xT = np.asarray(x, dtype=np.float32).reshape(s, MD).T  # [MD, s]
    # -> [128, n_mt, n_sc, SC] -> chunk-major [128, n_sc, n_mt*SC]
    x4 = xT.reshape(n_mt, PT, n_sc, SC).transpose(1, 2, 0, 3)
    xcp = np.ascontiguousarray(
        x4.reshape(PT, n_sc, n_mt * SC)).astype(BFNP)
    cosT = np.ascontiguousarray(
        np.asarray(cos, dtype=np.float32).T).astype(BFNP)
    sign = np.concatenate(
        [-np.ones((H // 2, 1)), np.ones((H // 2, 1))]).astype(np.float32)
    sinTs = np.ascontiguousarray(
        np.asarray(sin, dtype=np.float32).T * sign).astype(BFNP)
    wo = np.asarray(wo, dtype=np.float32)
    wq = np.asarray(wq, dtype=np.float32)
    wk = np.asarray(wk, dtype=np.float32)
    wv = np.asarray(wv, dtype=np.float32)
    in_maps = []
    for c in range(NCORES):
        in_maps.append({
            "xc": xcp,
            "wq": _pack_pm(np.ascontiguousarray(
                wq[:, :, c, :].reshape(MD, RH))),
            "wk": _pack_pm(np.ascontiguousarray(wk[:, c, :])),
            "wv": _pack_pm(np.ascontiguousarray(wv[:, c, :])),
            "wo": _pack_pm(np.ascontiguousarray(
                wo[:, c, :, :].reshape(RH, MD))),
            "cosT": cosT,
            "sinT": sinTs,
        })
    return in_maps


def unpack_out(outp_arr, s=S):
    """[128, s/128, MD] bf16 -> [s, MD] f32."""
    return np.ascontiguousarray(
        np.asarray(outp_arr).astype(np.float32).reshape(
            PT, s // PT, MD).transpose(1, 0, 2).reshape(s, MD))


_NC_CACHE = {}


def kernel(x, wq, wk, wv, wo, mask, sin, cos):
    s = x.shape[1]
    if s not in _NC_CACHE:
        _NC_CACHE[s] = build_bass(s)
    nc = _NC_CACHE[s]
    in_maps = shard_inputs(x, wq, wk, wv, wo, mask, sin, cos, s=s)
    res = run_bass_kernel_spmd(nc, in_maps, list(range(NCORES)))
    out = unpack_out(res.results[0]["outp"], s)
    for c in range(1, NCORES):
        out = out + unpack_out(res.results[c]["outp"], s)
    return out.reshape(1, s, MD).astype(np.float32)
